# revision 1
# baseline (speedup 1.0000x reference)
"""Two-layer GCN (PyG GCNConv x2 + gelu + scaled residual) on 8 trn2 NeuronCores.

Strategy (per the sharding hint):
  - Nodes partitioned contiguously across the 8 cores (12500 each); edges
    assigned to the core owning their destination node.
  - 128x128 weights replicated; per-layer node-feature tables (xw = x @ W)
    are computed shard-wise and AllGathered so each core can gather the
    rows of its edges' source nodes ("halo exchange" of remote features).
  - Per dst-block (128 nodes) aggregation: gather y[src] rows with the
    custom SWDGE dma_gather, build a weighted one-hot selection matrix
    S_w[k, j] = norm[k] * (dst_local[k] == j) on the vector engine, and
    accumulate z += S_w.T @ G on the tensor engine in PSUM.
  - Degree/normalization (segment sums over static graph structure) and
    edge->slot layout are host-side preprocessing of the static graph.

Math:
  gcn(x, W, b) = dinv * (segsum_dst(w_e * y[src]) + y[i]) + b,
      where y = (x @ W) * dinv, dinv = rsqrt(deg + 1)
  equivalently with host-computed norm_e = dinv[src]*w*dinv[dst]:
      agg[i] = segsum_dst(norm_e * xw[src]) + dinv2[i]*xw[i] + b
  h   = gelu(agg1) + (x @ (0.3*Wres) + 0.3*bres)
  out = gelu(agg2(h))
"""

import numpy as np

P = 128
D = 128
NCORES = 8
NPC = 12500          # nodes per core
NBLK = 98            # 128-node blocks per core (98*128 = 12544)
NPCP = NBLK * P      # padded nodes per core
TR = NCORES * NPCP   # table rows (100352)
NRANGE = 4
RSZ = TR // NRANGE   # 25088 rows per gather range (< int16 max)
MAX_CHUNKS_PER_CALL = 8   # dma_gather num_idxs cap = 1024 = 8*128

_CACHE = {}


def _preprocess(x, edge_index, edge_weight, W1, b1, W2, b2, Wres, bres):
    BN = NCORES * NPC
    src = np.asarray(edge_index[0], dtype=np.int64)
    dst = np.asarray(edge_index[1], dtype=np.int64)
    w = np.asarray(edge_weight, dtype=np.float64)

    deg = np.bincount(dst, weights=w, minlength=BN) + 1.0
    dinv = 1.0 / np.sqrt(deg)
    norm_e = (dinv[src] * w * dinv[dst]).astype(np.float32)
    dinv2 = (dinv * dinv).astype(np.float32)

    trow_src = (src // NPC) * NPCP + (src % NPC)      # table row of src node
    core = dst // NPC
    loc = dst - core * NPC
    blk = loc // P
    dl = (loc % P).astype(np.float32)
    rng = trow_src // RSZ
    ridx = (trow_src % RSZ).astype(np.int16)

    # group edges by (core, blk, rng); stable order within groups
    order = np.lexsort((rng, blk, core))
    core_s, blk_s, rng_s = core[order], blk[order], rng[order]
    # per-edge position within its (core, blk, rng) group
    gid = (core_s * NBLK + blk_s) * NRANGE + rng_s
    ngroups = NCORES * NBLK * NRANGE
    cnt = np.bincount(gid, minlength=ngroups)
    start = np.concatenate([[0], np.cumsum(cnt)[:-1]])
    q = np.arange(len(gid)) - start[gid]

    # global chunk counts per (blk, rng): max over cores
    cnt3 = cnt.reshape(NCORES, NBLK, NRANGE)
    K = np.ceil(cnt3 / P).astype(np.int64).max(axis=0)  # [NBLK, NRANGE]
    K = np.maximum(K, 1)
    ktot = K.sum(axis=1)                                 # chunks per block
    cbase = np.zeros((NBLK, NRANGE), np.int64)           # chunk col base
    run = 0
    for b in range(NBLK):
        for r in range(NRANGE):
            cbase[b, r] = run
            run += K[b, r]
    C = int(run)                                         # total chunk cols

    # slot for each edge: chunk col cc, partition p
    cc = cbase[blk_s, rng_s] + q // P
    pp = q % P

    idx16 = np.zeros((NCORES, P, 8 * C), np.int16)
    dstl = np.zeros((NCORES, P, C), np.float32)
    normv = np.zeros((NCORES, P, C), np.float32)
    ic = 8 * cc + pp // 16
    ir = pp % 16
    idx16[core_s, ir, ic] = ridx[order]
    dstl[core_s, pp, cc] = dl[order]
    normv[core_s, pp, cc] = norm_e[order]
    for g in range(1, 8):
        idx16[:, 16 * g:16 * g + 16, :] = idx16[:, 0:16, :]

    dinv2o = np.zeros((NCORES, P, NBLK), np.float32)
    d2 = np.zeros(NCORES * NPCP, np.float32)
    for c in range(NCORES):
        d2[c * NPCP:c * NPCP + NPC] = dinv2[c * NPC:(c + 1) * NPC]
    dinv2o = d2.reshape(NCORES, NBLK, P).transpose(0, 2, 1).copy()

    xT = np.zeros((NCORES, P, NPCP), np.float32)
    xf = np.asarray(x, dtype=np.float32)
    for c in range(NCORES):
        xT[c, :, :NPC] = xf[c * NPC:(c + 1) * NPC].T

    iota = np.broadcast_to(np.arange(P, dtype=np.float32), (P, P)).copy()
    ident = np.eye(P, dtype=np.float32)

    consts = dict(
        W1=np.asarray(W1, np.float32), W2=np.asarray(W2, np.float32),
        Wres03=(0.3 * np.asarray(Wres, np.float32)),
        iota=iota, ident=ident,
    )
    b1 = np.asarray(b1, np.float32)
    b2 = np.asarray(b2, np.float32)
    bres03 = 0.3 * np.asarray(bres, np.float32)
    has_b1 = bool(np.any(b1)) or bool(np.any(bres03))
    has_b2 = bool(np.any(b2))
    if has_b1:
        # pre-gelu bias b1 broadcast; post-gelu bres03 folded into xr add
        consts["bias1"] = np.broadcast_to(b1, (P, P)).copy()
        consts["bres03"] = np.broadcast_to(bres03, (P, P)).copy()
    if has_b2:
        consts["bias2"] = np.broadcast_to(b2, (P, P)).copy()

    in_maps = []
    for c in range(NCORES):
        m = dict(consts)
        m.update(xT=xT[c], idx16=idx16[c], dstl=dstl[c], normv=normv[c],
                 dinv2o=dinv2o[c])
        in_maps.append(m)
    return K, has_b1, has_b2, in_maps


def _build(K, has_b1, has_b2):
    import concourse.bacc as bacc
    import concourse.bass as bass
    import concourse.mybir as mybir
    import concourse.tile as tile

    f32 = mybir.dt.float32
    C = int(K.sum())
    nc = bacc.Bacc(num_swdge_queues=4)

    xT_d = nc.dram_tensor("xT", [P, NPCP], f32, kind="ExternalInput")
    W1_d = nc.dram_tensor("W1", [P, P], f32, kind="ExternalInput")
    W2_d = nc.dram_tensor("W2", [P, P], f32, kind="ExternalInput")
    Wres_d = nc.dram_tensor("Wres03", [P, P], f32, kind="ExternalInput")
    iota_d = nc.dram_tensor("iota", [P, P], f32, kind="ExternalInput")
    ident_d = nc.dram_tensor("ident", [P, P], f32, kind="ExternalInput")
    idx_d = nc.dram_tensor("idx16", [P, 8 * C], mybir.dt.int16, kind="ExternalInput")
    dstl_d = nc.dram_tensor("dstl", [P, C], f32, kind="ExternalInput")
    norm_d = nc.dram_tensor("normv", [P, C], f32, kind="ExternalInput")
    dinv2_d = nc.dram_tensor("dinv2o", [P, NBLK], f32, kind="ExternalInput")
    bias1_d = nc.dram_tensor("bias1", [P, P], f32, kind="ExternalInput") if has_b1 else None
    bres_d = nc.dram_tensor("bres03", [P, P], f32, kind="ExternalInput") if has_b1 else None
    bias2_d = nc.dram_tensor("bias2", [P, P], f32, kind="ExternalInput") if has_b2 else None

    out_d = nc.dram_tensor("out", [NPCP, D], f32, kind="ExternalOutput")

    xw1_own = nc.dram_tensor("xw1_own", [NPCP, D], f32)
    xr03_dr = nc.dram_tensor("xr03", [NPCP, D], f32)
    xw2_own = nc.dram_tensor("xw2_own", [NPCP, D], f32)
    table1 = nc.dram_tensor("table1", [TR, D], f32, addr_space="Shared")
    table2 = nc.dram_tensor("table2", [TR, D], f32, addr_space="Shared")

    rg = [list(range(NCORES))]

    with tile.TileContext(nc) as tc:
        with (
            tc.tile_pool(name="meta", bufs=1) as mp,
            tc.tile_pool(name="gp", bufs=6) as gp,
            tc.tile_pool(name="wk", bufs=3) as wk,
            tc.tile_pool(name="pz", bufs=2, space="PSUM") as pz,
            tc.tile_pool(name="pa", bufs=2, space="PSUM") as pa,
        ):
            # ---- resident tiles
            w1_t = mp.tile([P, P], f32)
            w2_t = mp.tile([P, P], f32)
            wr_t = mp.tile([P, P], f32)
            iota_t = mp.tile([P, P], f32)
            id_t = mp.tile([P, P], f32)
            idx_t = mp.tile([P, 8 * C], mybir.dt.int16)
            dstl_t = mp.tile([P, C], f32)
            norm_t = mp.tile([P, C], f32)
            dinv2_t = mp.tile([P, NBLK], f32)
            hT_t = mp.tile([P, NPCP], f32)
            nc.sync.dma_start(out=w1_t[:], in_=W1_d[:])
            nc.sync.dma_start(out=w2_t[:], in_=W2_d[:])
            nc.sync.dma_start(out=wr_t[:], in_=Wres_d[:])
            nc.sync.dma_start(out=iota_t[:], in_=iota_d[:])
            nc.sync.dma_start(out=id_t[:], in_=ident_d[:])
            nc.sync.dma_start(out=idx_t[:], in_=idx_d[:])
            nc.sync.dma_start(out=dstl_t[:], in_=dstl_d[:])
            nc.sync.dma_start(out=norm_t[:], in_=norm_d[:])
            nc.sync.dma_start(out=dinv2_t[:], in_=dinv2_d[:])
            if has_b1:
                bias1_t = mp.tile([P, P], f32)
                bres_t = mp.tile([P, P], f32)
                nc.sync.dma_start(out=bias1_t[:], in_=bias1_d[:])
                nc.sync.dma_start(out=bres_t[:], in_=bres_d[:])
            if has_b2:
                bias2_t = mp.tile([P, P], f32)
                nc.sync.dma_start(out=bias2_t[:], in_=bias2_d[:])

            # ---- phase A: xw1 = x@W1, xr03 = x@(0.3*Wres), shard-local
            for t in range(NBLK):
                xt = wk.tile([P, P], f32, tag="xt")
                nc.sync.dma_start(out=xt[:], in_=xT_d[:, t * P:(t + 1) * P])
                ps1 = pa.tile([P, P], f32, space="PSUM", tag="ps1")
                ps2 = pa.tile([P, P], f32, space="PSUM", tag="ps2")
                nc.tensor.matmul(ps1[:], xt[:], w1_t[:], start=True, stop=True)
                nc.tensor.matmul(ps2[:], xt[:], wr_t[:], start=True, stop=True)
                c1 = wk.tile([P, P], f32, tag="c1")
                c2 = wk.tile([P, P], f32, tag="c2")
                nc.vector.tensor_copy(out=c1[:], in_=ps1[:])
                if has_b1:
                    nc.vector.tensor_add(out=c2[:], in0=ps2[:], in1=bres_t[:])
                else:
                    nc.vector.tensor_copy(out=c2[:], in_=ps2[:])
                nc.sync.dma_start(out=xw1_own[t * P:(t + 1) * P, :], in_=c1[:])
                nc.sync.dma_start(out=xr03_dr[t * P:(t + 1) * P, :], in_=c2[:])

            nc.gpsimd.collective_compute(
                "AllGather", mybir.AluOpType.bypass, replica_groups=rg,
                ins=[xw1_own[:]], outs=[table1[:]],
            )

            # ---- per-layer edge aggregation pass
            def layer_pass(table_d, own_d, layer):
                cc = 0
                for b in range(NBLK):
                    zp = pz.tile([P, P], f32, space="PSUM", tag="z")
                    nch = int(K[b].sum())
                    ci = 0
                    for r in range(NRANGE):
                        kc = int(K[b, r])
                        j0 = 0
                        while j0 < kc:
                            ncall = min(MAX_CHUNKS_PER_CALL, kc - j0)
                            gb = gp.tile([P, ncall, D], f32, tag="g")
                            col0 = cc + j0
                            nc.gpsimd.dma_gather(
                                out_ap=gb[:],
                                in_ap=table_d[r * RSZ:(r + 1) * RSZ, :],
                                idxs_ap=idx_t[:, 8 * col0:8 * (col0 + ncall)],
                                num_idxs=P * ncall,
                                num_idxs_reg=P * ncall,
                                elem_size=D,
                                queue_num=r % 4,
                            )
                            for j in range(ncall):
                                col = col0 + j
                                sw = wk.tile([P, P], f32, tag="sw")
                                nc.vector.tensor_scalar(
                                    out=sw[:], in0=iota_t[:],
                                    scalar1=dstl_t[:, col:col + 1],
                                    scalar2=norm_t[:, col:col + 1],
                                    op0=mybir.AluOpType.is_equal,
                                    op1=mybir.AluOpType.mult,
                                )
                                nc.tensor.matmul(
                                    zp[:], sw[:], gb[:, j, :],
                                    start=(ci == 0), stop=(ci == nch - 1),
                                )
                                ci += 1
                            j0 += ncall
                        cc += kc
                    # epilogue
                    ob = wk.tile([P, P], f32, tag="ob")
                    nc.sync.dma_start(out=ob[:], in_=own_d[b * P:(b + 1) * P, :])
                    e1 = wk.tile([P, P], f32, tag="e1")
                    nc.vector.tensor_scalar(
                        out=e1[:], in0=ob[:],
                        scalar1=dinv2_t[:, b:b + 1], scalar2=None,
                        op0=mybir.AluOpType.mult,
                    )
                    e2 = wk.tile([P, P], f32, tag="e2")
                    nc.vector.tensor_add(out=e2[:], in0=zp[:], in1=e1[:])
                    if layer == 1 and has_b1:
                        nc.vector.tensor_add(out=e2[:], in0=e2[:], in1=bias1_t[:])
                    if layer == 2 and has_b2:
                        nc.vector.tensor_add(out=e2[:], in0=e2[:], in1=bias2_t[:])
                    ge = wk.tile([P, P], f32, tag="ge")
                    nc.scalar.activation(
                        out=ge[:], in_=e2[:],
                        func=mybir.ActivationFunctionType.Gelu,
                    )
                    if layer == 1:
                        xr = wk.tile([P, P], f32, tag="xr")
                        nc.sync.dma_start(out=xr[:], in_=xr03_dr[b * P:(b + 1) * P, :])
                        hb = wk.tile([P, P], f32, tag="hb")
                        nc.vector.tensor_add(out=hb[:], in0=ge[:], in1=xr[:])
                        pt = pz.tile([P, P], f32, space="PSUM", tag="pt")
                        nc.tensor.transpose(out=pt[:], in_=hb[:], identity=id_t[:])
                        nc.vector.tensor_copy(out=hT_t[:, b * P:(b + 1) * P], in_=pt[:])
                    else:
                        nc.sync.dma_start(out=out_d[b * P:(b + 1) * P, :], in_=ge[:])

            layer_pass(table1, xw1_own, 1)

            # ---- phase C: xw2 = h @ W2 (from SBUF-resident hT)
            for t in range(NBLK):
                psC = pa.tile([P, P], f32, space="PSUM", tag="ps1")
                nc.tensor.matmul(psC[:], hT_t[:, t * P:(t + 1) * P], w2_t[:],
                                 start=True, stop=True)
                cC = wk.tile([P, P], f32, tag="c1")
                nc.vector.tensor_copy(out=cC[:], in_=psC[:])
                nc.sync.dma_start(out=xw2_own[t * P:(t + 1) * P, :], in_=cC[:])

            nc.gpsimd.collective_compute(
                "AllGather", mybir.AluOpType.bypass, replica_groups=rg,
                ins=[xw2_own[:]], outs=[table2[:]],
            )

            layer_pass(table2, xw2_own, 2)

    nc.compile()
    return nc


def _get_compiled(K, has_b1, has_b2):
    key = (K.tobytes(), has_b1, has_b2)
    if key not in _CACHE:
        _CACHE[key] = _build(K, has_b1, has_b2)
    return _CACHE[key]


def kernel(x, edge_index, B, N, causal_edge_index, edge_weight,
           causal_edge_weight, W1, b1, W2, b2, Wres, bres):
    assert int(B) * int(N) == NCORES * NPC
    from concourse.bass_utils import run_bass_kernel_spmd

    K, has_b1, has_b2, in_maps = _preprocess(
        x, edge_index, edge_weight, W1, b1, W2, b2, Wres, bres)
    nc = _get_compiled(K, has_b1, has_b2)
    res = run_bass_kernel_spmd(nc, in_maps, list(range(NCORES)))
    out = np.concatenate(
        [res.results[c]["out"][:NPC] for c in range(NCORES)], axis=0)
    return out.astype(np.float32)


# exposed for test.py so it can reuse preprocessing + run with tracing
def _run_traced(x, edge_index, edge_weight, W1, b1, W2, b2, Wres, bres,
                **trace_kwargs):
    from concourse.bass_utils import run_bass_kernel_spmd
    K, has_b1, has_b2, in_maps = _preprocess(
        x, edge_index, edge_weight, W1, b1, W2, b2, Wres, bres)
    nc = _get_compiled(K, has_b1, has_b2)
    res = run_bass_kernel_spmd(nc, in_maps, list(range(NCORES)),
                               **trace_kwargs)
    out = np.concatenate(
        [res.results[c]["out"][:NPC] for c in range(NCORES)], axis=0)
    return out.astype(np.float32), res



# revision 2
# speedup vs baseline: 2.5238x; 2.5238x over previous
"""Two-layer GCN (PyG GCNConv x2 + gelu + scaled residual) on 8 trn2 NeuronCores.

Strategy (per the sharding hint):
  - Nodes partitioned contiguously across the 8 cores (12500 each); edges
    assigned to the core owning their destination node.
  - 128x128 weights replicated; per-layer node-feature tables (xw = x @ W)
    are computed shard-wise and AllGathered so each core can gather the
    rows of its edges' source nodes ("halo exchange" of remote features).
  - Per dst-block (128 nodes) aggregation: gather y[src] rows with the
    custom SWDGE dma_gather, build a weighted one-hot selection matrix
    S_w[k, j] = norm[k] * (dst_local[k] == j) on the vector engine, and
    accumulate z += S_w.T @ G on the tensor engine in PSUM.
  - Degree/normalization (segment sums over static graph structure) and
    edge->slot layout are host-side preprocessing of the static graph.

Math:
  gcn(x, W, b) = dinv * (segsum_dst(w_e * y[src]) + y[i]) + b,
      where y = (x @ W) * dinv, dinv = rsqrt(deg + 1)
  equivalently with host-computed norm_e = dinv[src]*w*dinv[dst]:
      agg[i] = segsum_dst(norm_e * xw[src]) + dinv2[i]*xw[i] + b
  h   = gelu(agg1) + (x @ (0.3*Wres) + 0.3*bres)
  out = gelu(agg2(h))
"""

import numpy as np

P = 128
D = 128
NCORES = 8
NPC = 12500          # nodes per core
NBLK = 98            # 128-node blocks per core (98*128 = 12544)
NPCP = NBLK * P      # padded nodes per core
TR = NCORES * NPCP   # table rows (100352)
NRANGE = 4
RSZ = TR // NRANGE   # 25088 rows per gather range (< int16 max)
MAX_CHUNKS_PER_CALL = 8   # dma_gather num_idxs cap = 1024 = 8*128

_CACHE = {}


def _preprocess(x, edge_index, edge_weight, W1, b1, W2, b2, Wres, bres):
    BN = NCORES * NPC
    src = np.asarray(edge_index[0], dtype=np.int64)
    dst = np.asarray(edge_index[1], dtype=np.int64)
    w = np.asarray(edge_weight, dtype=np.float64)

    deg = np.bincount(dst, weights=w, minlength=BN) + 1.0
    dinv = 1.0 / np.sqrt(deg)
    norm_e = (dinv[src] * w * dinv[dst]).astype(np.float32)
    dinv2 = (dinv * dinv).astype(np.float32)

    trow_src = (src // NPC) * NPCP + (src % NPC)      # table row of src node
    core = dst // NPC
    loc = dst - core * NPC
    blk = loc // P
    dl = (loc % P).astype(np.float32)
    rng = trow_src // RSZ
    ridx = (trow_src % RSZ).astype(np.int16)

    # group edges by (core, blk, rng); sort by src row within groups for
    # HBM row locality in the gather
    order = np.lexsort((trow_src, rng, blk, core))
    core_s, blk_s, rng_s = core[order], blk[order], rng[order]
    # per-edge position within its (core, blk, rng) group
    gid = (core_s * NBLK + blk_s) * NRANGE + rng_s
    ngroups = NCORES * NBLK * NRANGE
    cnt = np.bincount(gid, minlength=ngroups)
    start = np.concatenate([[0], np.cumsum(cnt)[:-1]])
    q = np.arange(len(gid)) - start[gid]

    # global chunk counts per (blk, rng): max over cores
    cnt3 = cnt.reshape(NCORES, NBLK, NRANGE)
    K = np.ceil(cnt3 / P).astype(np.int64).max(axis=0)  # [NBLK, NRANGE]
    K = np.maximum(K, 1)
    ktot = K.sum(axis=1)                                 # chunks per block
    cbase = np.zeros((NBLK, NRANGE), np.int64)           # chunk col base
    run = 0
    for b in range(NBLK):
        for r in range(NRANGE):
            cbase[b, r] = run
            run += K[b, r]
    C = int(run)                                         # total chunk cols

    # slot for each edge: chunk col cc, partition p
    cc = cbase[blk_s, rng_s] + q // P
    pp = q % P

    idx16 = np.zeros((NCORES, P, 8 * C), np.int16)
    dstl = np.zeros((NCORES, P, C), np.float32)
    normv = np.zeros((NCORES, P, C), np.float32)
    ic = 8 * cc + pp // 16
    ir = pp % 16
    idx16[core_s, ir, ic] = ridx[order]
    dstl[core_s, pp, cc] = dl[order]
    normv[core_s, pp, cc] = norm_e[order]
    for g in range(1, 8):
        idx16[:, 16 * g:16 * g + 16, :] = idx16[:, 0:16, :]

    dinv2o = np.zeros((NCORES, P, NBLK), np.float32)
    d2 = np.zeros(NCORES * NPCP, np.float32)
    for c in range(NCORES):
        d2[c * NPCP:c * NPCP + NPC] = dinv2[c * NPC:(c + 1) * NPC]
    dinv2o = d2.reshape(NCORES, NBLK, P).transpose(0, 2, 1).copy()

    xT = np.zeros((NCORES, P, NPCP), np.float32)
    xf = np.asarray(x, dtype=np.float32)
    for c in range(NCORES):
        xT[c, :, :NPC] = xf[c * NPC:(c + 1) * NPC].T

    iota = np.broadcast_to(np.arange(P, dtype=np.float32), (P, P)).copy()
    ident = np.eye(P, dtype=np.float32)

    consts = dict(
        W1=np.asarray(W1, np.float32), W2=np.asarray(W2, np.float32),
        Wres03=(0.3 * np.asarray(Wres, np.float32)),
        iota=iota, ident=ident,
    )
    b1 = np.asarray(b1, np.float32)
    b2 = np.asarray(b2, np.float32)
    bres03 = 0.3 * np.asarray(bres, np.float32)
    has_b1 = bool(np.any(b1)) or bool(np.any(bres03))
    has_b2 = bool(np.any(b2))
    if has_b1:
        # pre-gelu bias b1 broadcast; post-gelu bres03 folded into xr add
        consts["bias1"] = np.broadcast_to(b1, (P, P)).copy()
        consts["bres03"] = np.broadcast_to(bres03, (P, P)).copy()
    if has_b2:
        consts["bias2"] = np.broadcast_to(b2, (P, P)).copy()

    in_maps = []
    for c in range(NCORES):
        m = dict(consts)
        m.update(xT=xT[c], idx16=idx16[c], dstl=dstl[c], normv=normv[c],
                 dinv2o=dinv2o[c])
        in_maps.append(m)
    return K, has_b1, has_b2, in_maps


def _build(K, has_b1, has_b2):
    import concourse.bacc as bacc
    import concourse.bass as bass
    import concourse.mybir as mybir
    import concourse.tile as tile

    f32 = mybir.dt.float32
    C = int(K.sum())
    nc = bacc.Bacc(num_swdge_queues=4)

    xT_d = nc.dram_tensor("xT", [P, NPCP], f32, kind="ExternalInput")
    W1_d = nc.dram_tensor("W1", [P, P], f32, kind="ExternalInput")
    W2_d = nc.dram_tensor("W2", [P, P], f32, kind="ExternalInput")
    Wres_d = nc.dram_tensor("Wres03", [P, P], f32, kind="ExternalInput")
    iota_d = nc.dram_tensor("iota", [P, P], f32, kind="ExternalInput")
    ident_d = nc.dram_tensor("ident", [P, P], f32, kind="ExternalInput")
    idx_d = nc.dram_tensor("idx16", [P, 8 * C], mybir.dt.int16, kind="ExternalInput")
    dstl_d = nc.dram_tensor("dstl", [P, C], f32, kind="ExternalInput")
    norm_d = nc.dram_tensor("normv", [P, C], f32, kind="ExternalInput")
    dinv2_d = nc.dram_tensor("dinv2o", [P, NBLK], f32, kind="ExternalInput")
    bias1_d = nc.dram_tensor("bias1", [P, P], f32, kind="ExternalInput") if has_b1 else None
    bres_d = nc.dram_tensor("bres03", [P, P], f32, kind="ExternalInput") if has_b1 else None
    bias2_d = nc.dram_tensor("bias2", [P, P], f32, kind="ExternalInput") if has_b2 else None

    out_d = nc.dram_tensor("out", [NPCP, D], f32, kind="ExternalOutput")

    xw1_own = nc.dram_tensor("xw1_own", [NPCP, D], f32)
    xr03_dr = nc.dram_tensor("xr03", [NPCP, D], f32)
    xw2_own = nc.dram_tensor("xw2_own", [NPCP, D], f32)
    table1 = nc.dram_tensor("table1", [TR, D], f32, addr_space="Shared")
    table2 = nc.dram_tensor("table2", [TR, D], f32, addr_space="Shared")

    rg = [list(range(NCORES))]

    with tile.TileContext(nc) as tc:
        with (
            tc.tile_pool(name="meta", bufs=1) as mp,
            tc.tile_pool(name="gp", bufs=6) as gp,
            tc.tile_pool(name="wk", bufs=3) as wk,
            tc.tile_pool(name="pz", bufs=2, space="PSUM") as pz,
            tc.tile_pool(name="pa", bufs=2, space="PSUM") as pa,
        ):
            # ---- resident tiles
            w1_t = mp.tile([P, P], f32)
            w2_t = mp.tile([P, P], f32)
            wr_t = mp.tile([P, P], f32)
            iota_t = mp.tile([P, P], f32)
            id_t = mp.tile([P, P], f32)
            idx_t = mp.tile([P, 8 * C], mybir.dt.int16)
            dstl_t = mp.tile([P, C], f32)
            norm_t = mp.tile([P, C], f32)
            dinv2_t = mp.tile([P, NBLK], f32)
            hT_t = mp.tile([P, NPCP], f32)
            nc.sync.dma_start(out=w1_t[:], in_=W1_d[:])
            nc.sync.dma_start(out=w2_t[:], in_=W2_d[:])
            nc.sync.dma_start(out=wr_t[:], in_=Wres_d[:])
            nc.sync.dma_start(out=iota_t[:], in_=iota_d[:])
            nc.sync.dma_start(out=id_t[:], in_=ident_d[:])
            nc.sync.dma_start(out=idx_t[:], in_=idx_d[:])
            nc.sync.dma_start(out=dstl_t[:], in_=dstl_d[:])
            nc.sync.dma_start(out=norm_t[:], in_=norm_d[:])
            nc.sync.dma_start(out=dinv2_t[:], in_=dinv2_d[:])
            if has_b1:
                bias1_t = mp.tile([P, P], f32)
                bres_t = mp.tile([P, P], f32)
                nc.sync.dma_start(out=bias1_t[:], in_=bias1_d[:])
                nc.sync.dma_start(out=bres_t[:], in_=bres_d[:])
            if has_b2:
                bias2_t = mp.tile([P, P], f32)
                nc.sync.dma_start(out=bias2_t[:], in_=bias2_d[:])

            # ---- phase A: xw1 = x@W1, xr03 = x@(0.3*Wres), shard-local
            for t in range(NBLK):
                xt = wk.tile([P, P], f32, tag="xt")
                nc.sync.dma_start(out=xt[:], in_=xT_d[:, t * P:(t + 1) * P])
                ps1 = pa.tile([P, P], f32, space="PSUM", tag="ps1")
                ps2 = pa.tile([P, P], f32, space="PSUM", tag="ps2")
                nc.tensor.matmul(ps1[:], xt[:], w1_t[:], start=True, stop=True)
                nc.tensor.matmul(ps2[:], xt[:], wr_t[:], start=True, stop=True)
                c1 = wk.tile([P, P], f32, tag="c1")
                c2 = wk.tile([P, P], f32, tag="c2")
                nc.vector.tensor_copy(out=c1[:], in_=ps1[:])
                if has_b1:
                    nc.vector.tensor_add(out=c2[:], in0=ps2[:], in1=bres_t[:])
                else:
                    nc.vector.tensor_copy(out=c2[:], in_=ps2[:])
                nc.sync.dma_start(out=xw1_own[t * P:(t + 1) * P, :], in_=c1[:])
                nc.sync.dma_start(out=xr03_dr[t * P:(t + 1) * P, :], in_=c2[:])

            nc.gpsimd.collective_compute(
                "AllGather", mybir.AluOpType.bypass, replica_groups=rg,
                ins=[xw1_own[:]], outs=[table1[:]],
            )

            # ---- per-layer edge aggregation pass
            def layer_pass(table_d, own_d, layer):
                cc = 0
                for b in range(NBLK):
                    zp = pz.tile([P, P], f32, space="PSUM", tag="z")
                    nch = int(K[b].sum())
                    ci = 0
                    for r in range(NRANGE):
                        kc = int(K[b, r])
                        j0 = 0
                        while j0 < kc:
                            ncall = min(MAX_CHUNKS_PER_CALL, kc - j0)
                            gb = gp.tile([P, ncall, D], f32, tag="g")
                            col0 = cc + j0
                            nc.gpsimd.dma_gather(
                                out_ap=gb[:],
                                in_ap=table_d[r * RSZ:(r + 1) * RSZ, :],
                                idxs_ap=idx_t[:, 8 * col0:8 * (col0 + ncall)],
                                num_idxs=P * ncall,
                                num_idxs_reg=P * ncall,
                                elem_size=D,
                                queue_num=r % 4,
                            )
                            for j in range(ncall):
                                col = col0 + j
                                sw = wk.tile([P, P], f32, tag="sw")
                                nc.vector.tensor_scalar(
                                    out=sw[:], in0=iota_t[:],
                                    scalar1=dstl_t[:, col:col + 1],
                                    scalar2=norm_t[:, col:col + 1],
                                    op0=mybir.AluOpType.is_equal,
                                    op1=mybir.AluOpType.mult,
                                )
                                nc.tensor.matmul(
                                    zp[:], sw[:], gb[:, j, :],
                                    start=(ci == 0), stop=(ci == nch - 1),
                                )
                                ci += 1
                            j0 += ncall
                        cc += kc
                    # epilogue
                    ob = wk.tile([P, P], f32, tag="ob")
                    nc.sync.dma_start(out=ob[:], in_=own_d[b * P:(b + 1) * P, :])
                    e1 = wk.tile([P, P], f32, tag="e1")
                    nc.vector.tensor_scalar(
                        out=e1[:], in0=ob[:],
                        scalar1=dinv2_t[:, b:b + 1], scalar2=None,
                        op0=mybir.AluOpType.mult,
                    )
                    e2 = wk.tile([P, P], f32, tag="e2")
                    nc.vector.tensor_add(out=e2[:], in0=zp[:], in1=e1[:])
                    if layer == 1 and has_b1:
                        nc.vector.tensor_add(out=e2[:], in0=e2[:], in1=bias1_t[:])
                    if layer == 2 and has_b2:
                        nc.vector.tensor_add(out=e2[:], in0=e2[:], in1=bias2_t[:])
                    ge = wk.tile([P, P], f32, tag="ge")
                    nc.scalar.activation(
                        out=ge[:], in_=e2[:],
                        func=mybir.ActivationFunctionType.Gelu,
                    )
                    if layer == 1:
                        xr = wk.tile([P, P], f32, tag="xr")
                        nc.sync.dma_start(out=xr[:], in_=xr03_dr[b * P:(b + 1) * P, :])
                        hb = wk.tile([P, P], f32, tag="hb")
                        nc.vector.tensor_add(out=hb[:], in0=ge[:], in1=xr[:])
                        pt = pz.tile([P, P], f32, space="PSUM", tag="pt")
                        nc.tensor.transpose(out=pt[:], in_=hb[:], identity=id_t[:])
                        nc.vector.tensor_copy(out=hT_t[:, b * P:(b + 1) * P], in_=pt[:])
                    else:
                        nc.sync.dma_start(out=out_d[b * P:(b + 1) * P, :], in_=ge[:])

            layer_pass(table1, xw1_own, 1)

            # ---- phase C: xw2 = h @ W2 (from SBUF-resident hT)
            for t in range(NBLK):
                psC = pa.tile([P, P], f32, space="PSUM", tag="ps1")
                nc.tensor.matmul(psC[:], hT_t[:, t * P:(t + 1) * P], w2_t[:],
                                 start=True, stop=True)
                cC = wk.tile([P, P], f32, tag="c1")
                nc.vector.tensor_copy(out=cC[:], in_=psC[:])
                nc.sync.dma_start(out=xw2_own[t * P:(t + 1) * P, :], in_=cC[:])

            nc.gpsimd.collective_compute(
                "AllGather", mybir.AluOpType.bypass, replica_groups=rg,
                ins=[xw2_own[:]], outs=[table2[:]],
            )

            layer_pass(table2, xw2_own, 2)

    nc.compile()
    return nc


def _get_compiled(K, has_b1, has_b2):
    key = (K.tobytes(), has_b1, has_b2)
    if key not in _CACHE:
        _CACHE[key] = _build(K, has_b1, has_b2)
    return _CACHE[key]


def kernel(x, edge_index, B, N, causal_edge_index, edge_weight,
           causal_edge_weight, W1, b1, W2, b2, Wres, bres):
    assert int(B) * int(N) == NCORES * NPC
    from concourse.bass_utils import run_bass_kernel_spmd

    K, has_b1, has_b2, in_maps = _preprocess(
        x, edge_index, edge_weight, W1, b1, W2, b2, Wres, bres)
    nc = _get_compiled(K, has_b1, has_b2)
    res = run_bass_kernel_spmd(nc, in_maps, list(range(NCORES)))
    out = np.concatenate(
        [res.results[c]["out"][:NPC] for c in range(NCORES)], axis=0)
    return out.astype(np.float32)


# exposed for test.py so it can reuse preprocessing + run with tracing
def _run_traced(x, edge_index, edge_weight, W1, b1, W2, b2, Wres, bres,
                **trace_kwargs):
    from concourse.bass_utils import run_bass_kernel_spmd
    K, has_b1, has_b2, in_maps = _preprocess(
        x, edge_index, edge_weight, W1, b1, W2, b2, Wres, bres)
    nc = _get_compiled(K, has_b1, has_b2)
    res = run_bass_kernel_spmd(nc, in_maps, list(range(NCORES)),
                               **trace_kwargs)
    out = np.concatenate(
        [res.results[c]["out"][:NPC] for c in range(NCORES)], axis=0)
    return out.astype(np.float32), res

